# revision 9
# baseline (speedup 1.0000x reference)
"""Trainium2 Bass kernel: 1-D horizontal cost volume (9 disparities).

out[b, j, h, w] = mean_c( f1[b,c,h,w] * zeropad_w(f2)[b,c,h,w+j] ),  j = 0..8.

Sharding: 8 cores, each handles one (batch, H-half) slice [C=128, 96, W=640].
No halo needed (shift is along W only).

Per-core algorithm, per image row r (W split into 5 tiles of 128, each tile
split into 4 groups of 32 output columns):
  1. Band matmuls on TensorE: for tile t, group g,
       ps[b, 200g + 40t + c] = sum_ch f1[ch, 128t+32g+b]/C * f2pad[ch, 128t+32g+c]
     (lhsT = f1 cast to bf16 pre-scaled by 1/C, rhs = 40-col window of the
     zero-padded bf16 f2; output is a [32, 800] PSUM tile at base partition 0
     with the group index in the free dim, since matmul PSUM writes only
     support base partitions 0/32/64).  The 9 disparities of output column
     m = 128t+32g+b live at c = b+j, i.e. at in-window diagonals.
  2. The diagonal extraction must round-trip through DRAM (SBUF-side DMA
     access patterns cannot carry per-partition byte offsets), but both hops
     run at full DMA rate by absorbing the shear into DRAM base offsets:
     the park writes partition b's whole 1600-element row of the [32, 6400]
     bf16 window buffer CONTIGUOUSLY to
         X[31 + 32*A*g + (A-1)*b + (200*dr + 40t + c)],   A = 40*nu + 31,
     (one 3200B descriptor per (b, g); the -b shear is in the base term) so
     X[31 + A*m + 40u + j] = window col (b+j) of chunk u = diagonal j.  The
     readback D[m, :] = X[31 + A*m : + 1600] is then plain contiguous, and
     D[m, 200*dr + 40t + j] = out[j, r0+dr, 128t + m] for j < 9 uniformly
     across partitions (cols 9..39 of each 40-block are junk and skipped).
  3. PE-transposes [128, 9] -> [9, 128] per (row, tile) pack ps2[9, 640] =
     out[j, r, :] with j on partitions; upcast to fp32 into a per-block
     [9, nr, 640] buffer on Act/DVE; ONE out DMA per block (20KB runs).

Software pipeline (all DMA on SP, which also orders park -> readback without
a cross-queue semaphore stall): iteration rb emits
    loads(rb+1) | readback(rb-1) | casts(rb) | tail(rb-2): transposes +
    upcasts + 1 out DMA | matmuls+copies(rb) | park(rb)
so every DMA at the SP queue head has its dependencies satisfied ~a full
block before the DMA engines reach it.  Batched outs keep the exclusive
HWDGE setup device off the critical path (5 DMAs/block, not 12).  The last
two blocks are 4 rows to shorten the drain after the final input load.
"""

import numpy as np

import concourse.bass as bass
import concourse.bacc as bacc
import concourse.tile as tile
from concourse import mybir
from concourse import bass_utils
from concourse.masks import make_identity

B, C, H, W = 4, 128, 192, 640
NJ = 9                # 2*4+1 disparities
NCORES = 8
HS = B * H // NCORES  # 96 rows per core
NT = W // 128         # 5 w-tiles per row
NG = 8                # 16-column groups per tile
WIN = 24              # window columns per group (16 + 8 shear overhang)
ROWBLK = 8            # rows per pipeline block

F32 = mybir.dt.float32
BF16 = mybir.dt.bfloat16

_CACHE: dict = {}
TRACE = False  # set True (e.g. from test.py) to capture an NTFF profile
LAST_RESULT = None  # BassKernelResults of the most recent run when TRACE


def _block_sizes(hs: int):
    """All 8-row blocks except the last two (4 rows each) to shorten the
    post-last-load drain."""
    sizes = [ROWBLK] * (hs // ROWBLK - 1) + [4, 2, 1, 1]
    assert sum(sizes) == hs
    return sizes


def _build_program(hs: int = HS):
    from contextlib import ExitStack

    sizes = _block_sizes(hs)
    nblk = len(sizes)
    r0s = [sum(sizes[:i]) for i in range(nblk)]
    nc = bacc.Bacc("TRN2", target_bir_lowering=False, debug=False)
    f1 = nc.dram_tensor("f1", [C, hs, W], F32, kind="ExternalInput")
    f2 = nc.dram_tensor("f2", [C, hs, W], F32, kind="ExternalInput")
    out = nc.dram_tensor("out", [NJ, hs, W], F32, kind="ExternalOutput")

    with tile.TileContext(nc) as tc, ExitStack() as ctx:
        consts = ctx.enter_context(tc.tile_pool(name="consts", bufs=1))
        loads = ctx.enter_context(tc.tile_pool(name="loads", bufs=2))
        casts = ctx.enter_context(tc.tile_pool(name="casts", bufs=2))
        sb2p = ctx.enter_context(tc.tile_pool(name="sb2", bufs=2))
        d2p = ctx.enter_context(tc.tile_pool(name="d2", bufs=2))
        orowp = ctx.enter_context(tc.tile_pool(name="orow", bufs=2))
        ppool = ctx.enter_context(tc.tile_pool(name="psum1", bufs=3, space="PSUM"))
        ppool2 = ctx.enter_context(tc.tile_pool(name="psum2", bufs=2, space="PSUM"))
        xpool = ctx.enter_context(tc.tile_pool(name="xpark", bufs=2, space="DRAM"))

        ident = consts.tile([128, 128], BF16)
        make_identity(nc, ident)

        # f2 double buffer with the 4+4 zero pad columns written once.
        f2bufs = [
            consts.tile([128, ROWBLK, W + 8], BF16, name=f"f2buf{i}")
            for i in range(2)
        ]
        for fb in f2bufs:
            nc.gpsimd.memset(fb[:, :, 0:4], 0.0)
            nc.gpsimd.memset(fb[:, :, W + 4 : W + 8], 0.0)

        state: dict = {}

        def emit_loads(rb: int):
            r0, nr = r0s[rb], sizes[rb]
            f1row = loads.tile([128, nr, W], F32, name="f1row")
            nc.sync.dma_start(out=f1row, in_=f1.ap()[:, r0 : r0 + nr, :])
            f2row = loads.tile([128, nr, W], F32, name="f2row")
            nc.sync.dma_start(out=f2row, in_=f2.ap()[:, r0 : r0 + nr, :])
            state[("rows", rb)] = (f1row, f2row)

        def emit_readback(rb: int):
            """contiguous window readback of block rb (parked last iter)."""
            xblk = state.pop(("xblk", rb))
            nr = sizes[rb]  # noqa: F841
            apitch = WIN * NT * nr + (WIN - NJ)
            d2 = d2p.tile([128, nr, NT * WIN], BF16, name="d2")
            rsrc = bass.AP(
                xblk.tensor,
                xblk.offset + (WIN - NJ),
                [[apitch, 128], [1, nr * NT * WIN]],
            )
            q = nc.scalar if rb >= nblk - 3 else nc.sync
            q.dma_start(out=d2, in_=rsrc)
            state[("d2", rb)] = d2

        def emit_tail(rb: int):
            """transposes + fp32 upcast + one batched out DMA for block rb
            (readback of rb completed during the previous iteration)."""
            d2 = state.pop(("d2", rb))
            r0, nr = r0s[rb], sizes[rb]
            orow = orowp.tile([NJ, nr, NT * 128], F32, name="orow")
            for dr in range(nr):
                ps2 = ppool2.tile([NJ, NT * 128], BF16)
                for t in range(NT):
                    tsrc = bass.AP(
                        d2.tensor,
                        d2.offset + NT * WIN * dr + WIN * t,
                        [[nr * NT * WIN, 128], [1, NJ]],
                    )
                    nc.tensor.transpose(ps2[:, 128 * t : 128 * (t + 1)], tsrc, ident)
                if dr % 2 == 0:
                    nc.scalar.copy(orow[:, dr, :], ps2)
                else:
                    nc.vector.tensor_copy(orow[:, dr, :], ps2)
            odst = bass.AP(
                out.ap().tensor, r0 * W, [[hs * W, NJ], [W, nr], [1, NT * 128]]
            )
            oq = nc.scalar if rb >= nblk - 3 else nc.sync
            oq.dma_start(out=odst, in_=orow)

        emit_loads(0)
        for rb in range(nblk):
            r0, nr = r0s[rb], sizes[rb]
            nu = nr * NT
            apitch = WIN * nu + (WIN - NJ)  # per-partition DRAM park pitch
            # ---- prefetch next block's rows (SP queue: shallow deps) ----
            if rb + 1 < nblk:
                emit_loads(rb + 1)

            # ---- previous block's readback: its park sem fired during the
            # previous iteration, so the SP queue head never stalls on it
            # (final blocks already emitted theirs right after their park) ----
            if rb > 0 and rb - 1 < nblk - 3:
                emit_readback(rb - 1)

            # ---- cast to bf16 (f1 pre-scaled by 1/C); loads arrived during
            # the previous block, so these dispatch without waiting ----
            f1row, f2row = state.pop(("rows", rb))
            f1b = casts.tile([128, nr, W], BF16, name="f1b")
            nc.scalar.mul(f1b, f1row, 1.0 / C)
            f2b = f2bufs[rb % 2]
            nc.vector.tensor_copy(f2b[:, :nr, 4 : W + 4], f2row)

            # ---- two-blocks-ago tail: d2 arrived last iteration, so the PE
            # transposes dispatch before this block's matmuls without stalls
            # (final blocks run theirs right after their readback instead) ----
            if rb > 1 and rb - 2 < nblk - 3:
                emit_tail(rb - 2)

            # ---- banded matmuls: [32, 4*200] PSUM per row (group in the
            # free dim), then one copy into the bf16 window buffer ----
            gp = 128 // NG  # output columns per group
            sblk2 = sb2p.tile([gp, NG, nr, NT * WIN], BF16, name="sblk2")
            for dr in range(nr):
                ps = ppool.tile([gp, NG, NT * WIN], F32)
                for t in range(NT):
                    for g in range(NG):
                        w0 = 128 * t + gp * g
                        nc.tensor.matmul(
                            ps[:, g, WIN * t : WIN * (t + 1)],
                            f1b[:, dr, w0 : w0 + gp],
                            f2b[:, dr, w0 : w0 + WIN],
                            start=True,
                            stop=True,
                        )
                if dr % 2 == 0:
                    nc.vector.tensor_copy(sblk2[:, :, dr, :], ps)
                else:
                    nc.scalar.copy(sblk2[:, :, dr, :], ps)

            # ---- park: one DMA, 3200B contiguous runs; the diagonal shear
            # sits in the DRAM base term (A-1)*b ----
            xblk = xpool.tile([128, apitch], BF16, name="xblk")
            rowlen = nr * NT * WIN
            psrc = bass.AP(
                sblk2.tensor,
                sblk2.offset,
                [[NG * rowlen, gp], [rowlen, NG], [1, rowlen]],
            )
            pdst = bass.AP(
                xblk.tensor,
                xblk.offset + (WIN - NJ),
                [[apitch - 1, gp], [gp * apitch, NG], [1, rowlen]],
            )
            nc.sync.dma_start(out=pdst, in_=psrc)
            state[("xblk", rb)] = xblk
            # final blocks: readback immediately (queues are drained, so the
            # park->readback sem hop cannot head-of-line block anything) and
            # run their tails as soon as possible to overlap chain latencies
            if rb >= nblk - 3:
                emit_readback(rb)
                if rb >= nblk - 2:
                    emit_tail(rb - 1)

        emit_tail(nblk - 1)

    nc.compile()
    return nc


def _get_nc():
    if "nc" not in _CACHE:
        _CACHE["nc"] = _build_program()
    return _CACHE["nc"]


def kernel(feature1: np.ndarray, feature2: np.ndarray) -> np.ndarray:
    f1 = np.asarray(feature1, dtype=np.float32)
    f2 = np.asarray(feature2, dtype=np.float32)
    assert f1.shape == (B, C, H, W) and f2.shape == (B, C, H, W)

    nc = _get_nc()
    in_maps = []
    for core in range(NCORES):
        b = core // 2
        h0 = (core % 2) * HS
        in_maps.append(
            {
                "f1": np.ascontiguousarray(f1[b, :, h0 : h0 + HS, :]),
                "f2": np.ascontiguousarray(f2[b, :, h0 : h0 + HS, :]),
            }
        )
    try:
        res = bass_utils.run_bass_kernel_spmd(
            nc, in_maps, core_ids=list(range(NCORES)), trace=TRACE
        )
    except ModuleNotFoundError:
        if not TRACE:
            raise
        # NTFF profile hook unavailable (e.g. axon container): run untraced.
        res = bass_utils.run_bass_kernel_spmd(
            nc, in_maps, core_ids=list(range(NCORES)), trace=False
        )
    if TRACE:
        global LAST_RESULT
        LAST_RESULT = res
    outv = np.empty((B, NJ, H, W), dtype=np.float32)
    for core in range(NCORES):
        b = core // 2
        h0 = (core % 2) * HS
        outv[b, :, h0 : h0 + HS, :] = res.results[core]["out"]
    return outv


# revision 28
# speedup vs baseline: 1.0475x; 1.0475x over previous
"""Trainium2 Bass kernel: 1-D horizontal cost volume (9 disparities).

out[b, j, h, w] = mean_c( f1[b,c,h,w] * zeropad_w(f2)[b,c,h,w+j] ),  j = 0..8.

Sharding: 8 cores, each handles one (batch, H-half) slice [C=128, 96, W=640].
No halo needed (shift is along W only).

Per-core algorithm, per image row r (W split into 5 tiles of 128, each tile
split into 8 groups of 16 output columns):
  1. Band matmuls on TensorE: for tile t, group g,
       ps[b, 120g + 24t + c] = sum_ch f1b[ch, 128t+16g+b] * f2pad[ch, 128t+16g+c]
     (both inputs cast to bf16; output is a [16, 8*120] PSUM tile at base
     partition 0 with the group index in the free dim, since matmul PSUM
     writes only support base partitions 0/32/64).  The 9 disparities of
     output column m = 128t+16g+b live at window cols c = b+j.  One copy per
     row moves ps to the bf16 window buffer sblk2 [16, 8, nr, 120].
  2. The diagonal extraction must round-trip through DRAM (SBUF-side DMA
     access patterns cannot carry per-partition byte offsets - the BIR
     verifier rejects any partition step that is not a whole number of
     partitions), but both hops run at full DMA rate by absorbing the shear
     into DRAM base offsets: with A = 120*nr + 15, the park writes partition
     b of group g contiguously (one 2*120*nr-byte descriptor per (b, g)) to
         X[15 + 16*A*g + (A-1)*b + (120*dr + 24t + c)]
     so that X[15 + A*m + (120*dr + 24t + j)] = window col (b+j) of
     (row dr, tile t) = diagonal j for m = 16g+b.  The readback
     D[m, :] = X[15 + A*m :][: 120*nr] is then plain contiguous, and
     D[m, 120*dr + 24t + j] = out[j, r0+dr, 128t + m] UNIFORMLY across
     partitions (cols 9..23 of each 24-block are junk and get skipped).
  3. PE-transposes [128, 9] -> [9, 128] per (row, tile) pack ps2[9, 640] =
     out[:, r, :] with j on partitions; the fp32 upcast applies the deferred
     1/C channel-mean scale (exact power of two) into a per-block
     [9, nr, 640] buffer on Act/DVE; ONE out DMA per block (20KB runs).

Schedule: every DMA rides the in-order SP queue -- iteration rb emits
    loads(rb+1) | readback(rb-1) | casts(rb) | tail(rb-2): transposes +
    scaled upcasts + 1 out DMA | matmuls + copies(rb) | park(rb)
so each DMA at the queue head has had its dependencies satisfied for about a
full block by the time the DMA engines reach it: the cost model serializes
all transfers on one DMA-engines resource, so queue-head stalls directly
lengthen the run.  The readback lags its park by one iteration (the park's
completion semaphore fires under the next block's loads); the last two
readbacks ride the Act queue instead, since no later SP traffic covers their
semaphore wait.  Casts split: front rows on Act (f1) / DVE (f2), back rows
interleaved per-row on the otherwise-idle GPSIMD.  Batched per-block outs
keep the exclusive HWDGE setup device off the critical path.  Block sizes
taper [8 x 10, 6, 5, 4, 1] so the post-last-load drain works through
ever-smaller park/readback/tail chains.
"""

import numpy as np

import concourse.bass as bass
import concourse.bacc as bacc
import concourse.tile as tile
from concourse import mybir
from concourse import bass_utils
from concourse.masks import make_identity

B, C, H, W = 4, 128, 192, 640
NJ = 9                # 2*4+1 disparities
NCORES = 8
HS = B * H // NCORES  # 96 rows per core
NT = W // 128         # 5 w-tiles per row
NG = 16               # 8-column groups per tile
WIN = 16              # window columns per group (8 + 8 shear overhang)
ROWBLK = 8            # rows per pipeline block

F32 = mybir.dt.float32
BF16 = mybir.dt.bfloat16

_CACHE: dict = {}
TRACE = False  # set True (e.g. from test.py) to capture an NTFF profile
LAST_RESULT = None  # BassKernelResults of the most recent run when TRACE


def _block_sizes(hs: int):
    """8-row steady-state blocks with a tapered [6, 5, 4, 1] tail: the
    drain after the final input load then only chains small blocks."""
    sizes = [ROWBLK] * (hs // ROWBLK - 1) + [4, 2, 2]
    assert sum(sizes) == hs
    return sizes


def _build_program(hs: int = HS):
    from contextlib import ExitStack

    sizes = _block_sizes(hs)
    nblk = len(sizes)
    r0s = [sum(sizes[:i]) for i in range(nblk)]
    nc = bacc.Bacc("TRN2", target_bir_lowering=False, debug=False)
    f1 = nc.dram_tensor("f1", [C, hs, W], F32, kind="ExternalInput")
    f2 = nc.dram_tensor("f2", [C, hs, W], F32, kind="ExternalInput")
    out = nc.dram_tensor("out", [NJ, hs, W], F32, kind="ExternalOutput")

    with tile.TileContext(nc) as tc, ExitStack() as ctx:
        consts = ctx.enter_context(tc.tile_pool(name="consts", bufs=1))
        loads = ctx.enter_context(tc.tile_pool(name="loads", bufs=2))
        casts = ctx.enter_context(tc.tile_pool(name="casts", bufs=2))
        sb2p = ctx.enter_context(tc.tile_pool(name="sb2", bufs=2))
        d2p = ctx.enter_context(tc.tile_pool(name="d2", bufs=2))
        orowp = ctx.enter_context(tc.tile_pool(name="orow", bufs=2))
        ppool = ctx.enter_context(tc.tile_pool(name="psum1", bufs=2, space="PSUM"))
        ppool2 = ctx.enter_context(tc.tile_pool(name="psum2", bufs=2, space="PSUM"))
        xpool = ctx.enter_context(tc.tile_pool(name="xpark", bufs=2, space="DRAM"))

        ident = consts.tile([128, 128], BF16)

        # f2 double buffer with the 4+4 zero pad columns written once.
        f2bufs = [
            consts.tile([128, ROWBLK, W + 8], BF16, name=f"f2buf{i}")
            for i in range(2)
        ]
        state: dict = {}

        def emit_prologue():
            make_identity(nc, ident)
            for fb in f2bufs:
                nc.gpsimd.memset(fb[:, :, 0:4], 0.0)
                nc.gpsimd.memset(fb[:, :, W + 4 : W + 8], 0.0)

        def emit_loads(rb: int):
            r0, nr = r0s[rb], sizes[rb]
            f1row = loads.tile([128, nr, W], F32, name="f1row")
            nc.sync.dma_start(out=f1row, in_=f1.ap()[:, r0 : r0 + nr, :])
            f2row = loads.tile([128, nr, W], F32, name="f2row")
            nc.sync.dma_start(out=f2row, in_=f2.ap()[:, r0 : r0 + nr, :])
            state[("rows", rb)] = (f1row, f2row)

        def emit_readback(rb: int):
            """contiguous window readback of block rb (parked last iter)."""
            xblk = state.pop(("xblk", rb))
            nr = sizes[rb]  # noqa: F841
            apitch = WIN * NT * nr + (WIN - NJ)
            d2 = d2p.tile([128, nr, NT * WIN], BF16, name="d2")
            rsrc = bass.AP(
                xblk.tensor,
                xblk.offset + (WIN - NJ),
                [[apitch, 128], [1, nr * NT * WIN]],
            )
            q = nc.scalar if rb >= nblk - 2 else nc.sync
            q.dma_start(out=d2, in_=rsrc)
            state[("d2", rb)] = d2

        def emit_tail(rb: int):
            """transposes + fp32 upcast + one batched out DMA for block rb
            (readback of rb completed during the previous iteration)."""
            d2 = state.pop(("d2", rb))
            r0, nr = r0s[rb], sizes[rb]
            orow = orowp.tile([NJ, nr, NT * 128], F32, name="orow")
            for dr in range(nr):
                ps2 = ppool2.tile([NJ, NT * 128], BF16)
                for t in range(NT):
                    tsrc = bass.AP(
                        d2.tensor,
                        d2.offset + NT * WIN * dr + WIN * t,
                        [[nr * NT * WIN, 128], [1, NJ]],
                    )
                    nc.tensor.transpose(ps2[:, 128 * t : 128 * (t + 1)], tsrc, ident)
                if dr % 2 == 0:
                    nc.scalar.mul(orow[:, dr, :], ps2, 1.0 / C)
                else:
                    nc.vector.tensor_scalar_mul(orow[:, dr, :], ps2, 1.0 / C)
            odst = bass.AP(
                out.ap().tensor, r0 * W, [[hs * W, NJ], [W, nr], [1, NT * 128]]
            )
            nc.sync.dma_start(out=odst, in_=orow)

        emit_loads(0)
        emit_prologue()
        for rb in range(nblk):
            r0, nr = r0s[rb], sizes[rb]
            nu = nr * NT
            apitch = WIN * nu + (WIN - NJ)  # per-partition DRAM park pitch
            # ---- prefetch next block's rows (SP queue: shallow deps) ----
            if rb + 1 < nblk:
                emit_loads(rb + 1)

            # ---- previous block's readback: its park sem fired during the
            # previous iteration, so the SP queue head never stalls on it
            # (final blocks already emitted theirs right after their park) ----
            if rb > 0:
                emit_readback(rb - 1)

            # ---- cast to bf16 (f1 pre-scaled by 1/C); loads arrived during
            # the previous block, so these dispatch without waiting ----
            f1row, f2row = state.pop(("rows", rb))
            f1b = casts.tile([128, nr, W], BF16, name="f1b")
            nc.scalar.mul(f1b, f1row, 1.0 / C)
            f2b = f2bufs[rb % 2]
            nc.gpsimd.tensor_copy(f2b[:, :nr, 4 : W + 4], f2row)

            # ---- two-blocks-ago tail: d2 arrived last iteration, so the PE
            # transposes dispatch before this block's matmuls without stalls
            # (final blocks run theirs right after their readback instead) ----
            if rb > 1:
                emit_tail(rb - 2)

            # ---- banded matmuls: [32, 4*200] PSUM per row (group in the
            # free dim), then one copy into the bf16 window buffer ----
            gp = 128 // NG  # output columns per group
            sblk2 = sb2p.tile([gp, NG, nr, NT * WIN], BF16, name="sblk2")
            for dr in range(nr):
                ps = ppool.tile([gp, NG, NT * WIN], F32)
                for t in range(NT):
                    for g in range(NG):
                        w0 = 128 * t + gp * g
                        nc.tensor.matmul(
                            ps[:, g, WIN * t : WIN * (t + 1)],
                            f1b[:, dr, w0 : w0 + gp],
                            f2b[:, dr, w0 : w0 + WIN],
                            start=True,
                            stop=True,
                        )
                if dr % 2 == 0:
                    nc.vector.tensor_copy(sblk2[:, :, dr, :], ps)
                else:
                    nc.scalar.copy(sblk2[:, :, dr, :], ps)

            # ---- park: one DMA, 3200B contiguous runs; the diagonal shear
            # sits in the DRAM base term (A-1)*b ----
            xblk = xpool.tile([128, apitch], BF16, name="xblk")
            rowlen = nr * NT * WIN
            psrc = bass.AP(
                sblk2.tensor,
                sblk2.offset,
                [[NG * rowlen, gp], [rowlen, NG], [1, rowlen]],
            )
            pdst = bass.AP(
                xblk.tensor,
                xblk.offset + (WIN - NJ),
                [[apitch - 1, gp], [gp * apitch, NG], [1, rowlen]],
            )
            nc.sync.dma_start(out=pdst, in_=psrc)
            state[("xblk", rb)] = xblk

        emit_readback(nblk - 1)
        emit_tail(nblk - 2)
        emit_tail(nblk - 1)

    nc.compile()
    return nc


def _get_nc():
    if "nc" not in _CACHE:
        _CACHE["nc"] = _build_program()
    return _CACHE["nc"]


def kernel(feature1: np.ndarray, feature2: np.ndarray) -> np.ndarray:
    f1 = np.asarray(feature1, dtype=np.float32)
    f2 = np.asarray(feature2, dtype=np.float32)
    assert f1.shape == (B, C, H, W) and f2.shape == (B, C, H, W)

    nc = _get_nc()
    in_maps = []
    for core in range(NCORES):
        b = core // 2
        h0 = (core % 2) * HS
        in_maps.append(
            {
                "f1": np.ascontiguousarray(f1[b, :, h0 : h0 + HS, :]),
                "f2": np.ascontiguousarray(f2[b, :, h0 : h0 + HS, :]),
            }
        )
    try:
        res = bass_utils.run_bass_kernel_spmd(
            nc, in_maps, core_ids=list(range(NCORES)), trace=TRACE
        )
    except ModuleNotFoundError:
        if not TRACE:
            raise
        # NTFF profile hook unavailable (e.g. axon container): run untraced.
        res = bass_utils.run_bass_kernel_spmd(
            nc, in_maps, core_ids=list(range(NCORES)), trace=False
        )
    if TRACE:
        global LAST_RESULT
        LAST_RESULT = res
    outv = np.empty((B, NJ, H, W), dtype=np.float32)
    for core in range(NCORES):
        b = core // 2
        h0 = (core % 2) * HS
        outv[b, :, h0 : h0 + HS, :] = res.results[core]["out"]
    return outv
